# revision 6
# baseline (speedup 1.0000x reference)
"""Trainium2 Bass kernel for nn_ContrastiveDist (supervised contrastive loss).

Math
----
The reference builds (n,n) distance/weight matrices, but the loss collapses
exactly to per-class statistics.  With classes c = 0..15, per-class count
cnt[c], feature sums C[c,:], squared-norm sums SqSum[c], global sums
Ftot / SSall:

    alpha[c] = 1/(cnt[c]-1+eps)
    beta[c]  = 1/(n-cnt[c]+eps)
    loss_i   = sq_i*P[c_i] + (Q[c_i]+M) + f_i . R[c_i]
      P[c]   = alpha*cnt - beta*(n-cnt)
      Q[c]   = alpha*SqSum[c] - beta*(SSall-SqSum[c])
      R[c,:] = 2*beta*(Ftot-C[c]) - 2*alpha*C[c]
    result   = sum(relu(loss_i)*valid_i) / max(sum(valid_i), 1)

valid_i = (cnt[c_i] >= 2) is folded into the coefficients: Raug rows of
invalid classes are zeroed, so relu(loss) = 0 there, and the valid count
comes from sum(cnt[c]*vmask[c]).

Precision: everything on the feature path is bf16 (single chain, no hi/lo
split) with fp32 PSUM accumulation; numpy pipeline sim puts the result at
rel err <= ~6e-4 vs the fp32 reference (harness gate is 2e-2).

v2 vs baseline: one bf16 copy of F instead of fp32 + bf16 hi/lo (3.4x less
HBM), one-hots precomputed on host (no DVE is_equal), sq computed from bf16
via chunked Square+reduce overlapped with the load, single-chain matmuls
everywhere.  Every core redundantly computes the full loss.
"""

import numpy as np
import ml_dtypes

import concourse.bacc as bacc
import concourse.tile as tile
import concourse.mybir as mybir
from concourse.bass_utils import run_bass_kernel_spmd

N, D, K, NCORES = 8192, 128, 16, 8
T = N // 128               # 64 row-tiles of 128
W = D + 3                  # faug stride: [F(128), sq, 1, pad]
CHUNK = 8                  # row-tiles per DMA/compute chunk
EPS, MARGIN = 1e-6, 10.0
F32 = mybir.dt.float32
BF16 = mybir.dt.bfloat16
Alu = mybir.AluOpType
Act = mybir.ActivationFunctionType
AxX = mybir.AxisListType.X

# const tensor (128, CW) f32:
#   col 0       1.0 (ones(128,1) lhsT for the final partition reduce)
#   cols 1:17   1.0 in rows 0:16 (ones(16,16) lhsT for global-sum broadcast)
CW = 18

_CACHE: dict = {}


def _build():
    if "nc" in _CACHE:
        return _CACHE["nc"]

    nc = bacc.Bacc("TRN2", target_bir_lowering=False, debug=False, num_devices=NCORES)
    fbin = nc.dram_tensor("fbin", [128, T * W], BF16, kind="ExternalInput").ap()
    eohin = nc.dram_tensor("eohin", [128, T * 16], BF16, kind="ExternalInput").ap()
    eohTin = nc.dram_tensor("eohTin", [16, N], BF16, kind="ExternalInput").ap()
    cst = nc.dram_tensor("cst", [128, CW], F32, kind="ExternalInput").ap()
    res = nc.dram_tensor("res", [1, 1], F32, kind="ExternalOutput").ap()

    with tile.TileContext(nc) as tc:
        with (
            tc.tile_pool(name="sb", bufs=1) as sb,
            tc.tile_pool(name="ps", bufs=1, space="PSUM") as ps,
        ):
            # ---------------- loads ----------------
            # faug chunks round-robin over 4 engine queues so they land
            # earliest; one-hots/consts on the gpsimd queue.
            csts = sb.tile([128, CW], F32)
            nc.gpsimd.dma_start(csts[:], cst)
            eoh = sb.tile([128, T * 16], BF16)
            nc.gpsimd.dma_start(eoh[:], eohin)
            eohT = sb.tile([16, N], BF16)
            nc.gpsimd.dma_start(eohT[:], eohTin)

            faug = sb.tile([128, T * W], BF16)
            fa3 = faug.rearrange("p (t w) -> p t w", w=W)
            NCH = T // CHUNK
            CWID = CHUNK * W
            qengs = [nc.sync, nc.scalar]
            for g in range(NCH):
                eng = qengs[g % len(qengs)]
                eng.dma_start(faug[:, g * CWID:(g + 1) * CWID],
                              fbin[:, g * CWID:(g + 1) * CWID])

            eoh3 = eoh.rearrange("p (t c) -> p t c", c=16)

            # ---------- sq + per-class stats, chunk-pipelined with load ----
            sqd = sb.tile([128, T], F32)
            statsP = ps.tile([16, D + 2], F32)
            for g in range(NCH):
                t0 = g * CHUNK
                scr = sb.tile([128, CHUNK * D], BF16, tag="sqscr", bufs=2,
                              name=f"scr{g}")
                scr3 = scr.rearrange("p (t d) -> p t d", d=D)
                nc.scalar.activation(scr3[:, :, :], fa3[:, t0:t0 + CHUNK, 0:D],
                                     Act.Square)
                nc.vector.tensor_reduce(sqd[:, t0:t0 + CHUNK], scr3,
                                        axis=AxX, op=Alu.add)
                nc.vector.tensor_copy(fa3[:, t0:t0 + CHUNK, D],
                                      sqd[:, t0:t0 + CHUNK])
                for j in range(CHUNK):
                    t = t0 + j
                    nc.tensor.matmul(statsP[:], eoh3[:, t, :],
                                     fa3[:, t, 0:D + 2],
                                     start=(t == 0), stop=(t == T - 1))
            stats = sb.tile([16, D + 2], F32)
            nc.vector.tensor_copy(stats[:], statsP[:])

            # ---------------- per-class coefficients ----------------
            C = stats[:, 0:D]
            SqS = stats[:, D:D + 1]
            cnt = stats[:, D + 1:D + 2]
            gbP = ps.tile([16, D + 2], F32)
            nc.tensor.matmul(gbP[:], csts[0:16, 1:17], stats[:],
                             start=True, stop=True)
            gb = sb.tile([16, D + 2], F32)
            nc.vector.tensor_copy(gb[:], gbP[:])
            Ftot = gb[:, 0:D]
            SSall = gb[:, D:D + 1]

            alpha = sb.tile([16, 1], F32)
            nc.vector.tensor_scalar(alpha[:], cnt, EPS - 1.0, None, op0=Alu.add)
            nc.vector.reciprocal(alpha[:], alpha[:])
            beta = sb.tile([16, 1], F32)
            nc.vector.tensor_scalar(beta[:], cnt, -1.0, float(N) + EPS,
                                    op0=Alu.mult, op1=Alu.add)
            nc.vector.reciprocal(beta[:], beta[:])
            nalpha2 = sb.tile([16, 1], F32)
            nc.vector.tensor_scalar(nalpha2[:], alpha[:], -2.0, None, op0=Alu.mult)
            beta2 = sb.tile([16, 1], F32)
            nc.vector.tensor_scalar(beta2[:], beta[:], 2.0, None, op0=Alu.mult)

            raug = sb.tile([16, D + 2], F32)
            tmpd = sb.tile([16, D], F32)
            nc.vector.tensor_tensor(tmpd[:], Ftot, C, op=Alu.subtract)
            nc.vector.tensor_scalar(tmpd[:], tmpd[:], beta2[:], None, op0=Alu.mult)
            nc.vector.scalar_tensor_tensor(raug[:, 0:D], C, nalpha2[:], tmpd[:],
                                           op0=Alu.mult, op1=Alu.add)
            nmc = sb.tile([16, 1], F32)
            nc.vector.tensor_scalar(nmc[:], cnt, -1.0, float(N),
                                    op0=Alu.mult, op1=Alu.add)
            nc.vector.tensor_tensor(nmc[:], nmc[:], beta[:], op=Alu.mult)
            nc.vector.scalar_tensor_tensor(raug[:, D:D + 1], cnt, alpha[:], nmc[:],
                                           op0=Alu.mult, op1=Alu.subtract)
            ssd = sb.tile([16, 1], F32)
            nc.vector.tensor_tensor(ssd[:], SSall, SqS, op=Alu.subtract)
            nc.vector.tensor_tensor(ssd[:], ssd[:], beta[:], op=Alu.mult)
            qa = sb.tile([16, 1], F32)
            nc.vector.scalar_tensor_tensor(qa[:], SqS, alpha[:], ssd[:],
                                           op0=Alu.mult, op1=Alu.subtract)
            nc.vector.tensor_scalar(raug[:, D + 1:D + 2], qa[:], MARGIN, None,
                                    op0=Alu.add)

            # fold validity into the coefficients: zero Raug rows of classes
            # with cnt < 2, so relu(loss) vanishes for invalid rows
            vmask = sb.tile([16, 1], F32)
            nc.vector.tensor_scalar(vmask[:], cnt, 1.5, None, op0=Alu.is_ge)
            nc.vector.tensor_scalar(raug[:], raug[:], vmask[:], None, op0=Alu.mult)

            rhi = sb.tile([16, D + 2], BF16)
            nc.vector.tensor_copy(rhi[:], raug[:])

            # ---------------- per-row losses ----------------
            # dot ops split: every 4th tile's dot runs on gpsimd to offload DVE
            lossrows = sb.tile([128, T], F32)
            for t in range(T):
                dP = ps.tile([128, 512], F32, tag="dpsum", bufs=4, name=f"dP{t}")
                nc.tensor.matmul(dP[:, 0:D + 2], eohT[:, t * 128:(t + 1) * 128],
                                 rhi[:], start=True, stop=True)
                pscr = sb.tile([128, D + 2], F32, tag="pscr", bufs=4,
                               name=f"pt{t}")
                nc.vector.scalar_tensor_tensor(
                    pscr[:], dP[:, 0:D + 2], 0.0, fa3[:, t, 0:D + 2],
                    op0=Alu.bypass, op1=Alu.mult,
                    accum_out=lossrows[:, t:t + 1])

            # ---------------- final reduction ----------------
            accpair = sb.tile([128, 2], F32)
            nc.gpsimd.memset(accpair[:, 1:2], 0.0)
            relscr = sb.tile([128, T], F32)
            nc.vector.tensor_scalar(relscr[:], lossrows[:], 0.0, None,
                                    op0=Alu.max, op1=Alu.add,
                                    accum_out=accpair[:, 0:1])
            nc.vector.tensor_tensor(accpair[0:16, 1:2], cnt, vmask[:],
                                    op=Alu.mult)
            finP = ps.tile([1, 2], F32)
            nc.tensor.matmul(finP[:], csts[:, 0:1], accpair[:],
                             start=True, stop=True)
            fin = sb.tile([1, 2], F32)
            nc.vector.tensor_copy(fin[:], finP[:])
            den = sb.tile([1, 1], F32)
            nc.vector.tensor_scalar(den[:], fin[:, 1:2], 1.0, None, op0=Alu.max)
            nc.vector.reciprocal(den[:], den[:])
            resS = sb.tile([1, 1], F32)
            nc.vector.tensor_tensor(resS[:], fin[:, 0:1], den[:], op=Alu.mult)
            nc.sync.dma_start(res, resS[:])

    nc.compile()
    _CACHE["nc"] = nc
    return nc


def _make_in_maps(features, labels):
    feats = np.ascontiguousarray(np.asarray(features, dtype=np.float32))
    lab = np.ascontiguousarray(np.asarray(labels)).astype(np.int64)

    cst = np.zeros((128, CW), np.float32)
    cst[:, 0] = 1.0
    cst[0:16, 1:17] = 1.0

    fa = np.zeros((128, T, W), np.float32)
    fa[:, :, 0:D] = feats.reshape(T, 128, D).transpose(1, 0, 2)
    fa[:, :, D + 1] = 1.0
    fb = fa.reshape(128, T * W).astype(ml_dtypes.bfloat16)

    labT = lab.reshape(T, 128).T                       # (128, T)
    eoh = (labT[:, :, None] == np.arange(16)[None, None, :])
    eohin = np.ascontiguousarray(
        eoh.reshape(128, T * 16)).astype(ml_dtypes.bfloat16)
    eohT = (lab[None, :] == np.arange(16)[:, None])
    eohTin = np.ascontiguousarray(eohT).astype(ml_dtypes.bfloat16)

    one = {
        "fbin": fb,
        "eohin": eohin,
        "eohTin": eohTin,
        "cst": cst,
    }
    return [dict(one) for _ in range(NCORES)]


def kernel(features, labels):
    nc = _build()
    in_maps = _make_in_maps(features, labels)
    out = run_bass_kernel_spmd(nc, in_maps, core_ids=list(range(NCORES)))
    return np.float32(out.results[0]["res"][0, 0])
